# revision 22
# baseline (speedup 1.0000x reference)
"""Distributed Trainium2 Bass kernel for multi-head attention.

Problem: b=2, n=2048, dim=1024, heads=16, head_dim=64 (inner=1024), f32 I/O.

Sharding (Megatron-style, per the hint): data-parallel over batch (cores 0-3
handle batch 0, cores 4-7 batch 1) x tensor-parallel over heads (core c%4
owns heads 4*(c%4)..4*(c%4)+3 via column shards of Wq/Wk/Wv and row shards
of Wo). Each core produces a partial [n, dim] output (its 4 heads pushed
through its Wo row block); the unshard step sums the 4 partials per batch
(the "all-reduce after to_out" done at gather time -- measured on this fleet,
the on-device collective is ~60us/MB which would dominate the compute).

Per-core device pipeline (all matmuls bf16, f32 PSUM accumulation):
  1. qpT/kpT = Wq^T q^T etc in transposed [inner_loc, n] layout; vp in
     natural [n, inner_loc] layout padded with a ones column per head
     (so P@V also yields the softmax denominator for free as row 64).
  2. S^T = kh qh^T per head in [n_k, n_q] layout; exp on ScalarE with the
     1/sqrt(dh) scale folded into the activation; no max-subtraction
     (scores are ~N(0,1), exp is safe in f32).
  3. O^T (+denominator row) accumulated in PSUM over n_k tiles.
  4. Per n_q chunk: reciprocal of denominators, broadcast via a tiny
     mask-matmul, normalize O^T tiles, then the Wo projection emits the
     final [n_q, dim] rows in natural layout.

Scheduling: a dummy matmul burst warms the PE clock during the input DMAs;
K/Q projections for head-pair 0 run first so the chunk-0 softmax stream
starts on ScalarE as early as possible (its exp batches are deferred from
their P@V consumers to buy run-ahead); the V projection and the Wo
projection overlap the ScalarE-bound attention stream.
"""

import sys

if "/opt/trn_rl_repo" not in sys.path:
    sys.path.insert(0, "/opt/trn_rl_repo")

import numpy as np
import ml_dtypes

import concourse.bass as bass
import concourse.mybir as mybir
from concourse import bacc, tile
from concourse.bass_utils import run_bass_kernel_spmd
from concourse.masks import make_identity

BF16 = mybir.dt.bfloat16
F32 = mybir.dt.float32
NPBF16 = ml_dtypes.bfloat16

B = 2
N = 2048          # sequence length (full, per batch)
D = 1024          # model dim
H = 16            # total heads
DH = 64           # head dim
H_LOC = 4         # heads per core
INNER = H_LOC * DH  # 256, local inner dim
KC = D // 128     # 8 contraction chunks over model dim
KT = N // 128     # 16 k-tiles over sequence
NQC = N // 512    # 4 query chunks of 512
SCALE = DH ** -0.5
ES_BUFS = 36      # es slot pool (shared with the q input tiles)


def _build_nc():
    nc = bacc.Bacc("TRN2", target_bir_lowering=False, debug=False, num_devices=8)

    qT = nc.declare_dram_parameter("qT", [D, N], BF16, isOutput=False)
    kT = nc.declare_dram_parameter("kT", [D, N], BF16, isOutput=False)
    vT = nc.declare_dram_parameter("vT", [D, N], BF16, isOutput=False)
    wq = nc.declare_dram_parameter("wq", [D, INNER], BF16, isOutput=False)
    wk = nc.declare_dram_parameter("wk", [D, INNER], BF16, isOutput=False)
    wv = nc.declare_dram_parameter("wv", [D, INNER], BF16, isOutput=False)
    wo = nc.declare_dram_parameter("wo", [INNER, D], BF16, isOutput=False)
    emask = nc.declare_dram_parameter("emask", [4, 256], BF16, isOutput=False)
    out = nc.declare_dram_parameter("out", [N, D], F32, isOutput=True)

    with tile.TileContext(nc) as tc:
        with (
            tc.tile_pool(name="persist", bufs=1) as pp,
            tc.tile_pool(name="xkv", bufs=10) as xkv,
            tc.tile_pool(name="work", bufs=2) as wk_pool,
            tc.tile_pool(name="psum", bufs=2, space="PSUM") as psum,
        ):
            # ---- ScalarE exp table preload + PE clock warm-up burst
            warm = pp.tile([1, 16], F32, tag="warm", name="warm")
            nc.vector.memset(warm[:], 0.0)
            nc.scalar.activation(warm[:], warm[:], mybir.ActivationFunctionType.Exp)
            wa = pp.tile([128, 16], BF16, tag="wa", name="wa")
            wr = pp.tile([128, 512], BF16, tag="wr", name="wr")
            nc.vector.memset(wa[:], 0.0)
            nc.vector.memset(wr[:], 0.0)
            for i in range(10):
                wps = psum.tile([16, 512], F32, tag="epi", name="wps", bufs=2)
                nc.tensor.matmul(wps[:], lhsT=wa[:], rhs=wr[:], start=True, stop=True)

            # ---- persistent weight tiles
            wq_sb = [pp.tile([128, INNER], BF16, tag=f"wq{k}", name=f"wq{k}") for k in range(KC)]
            wk_sb = [pp.tile([128, INNER], BF16, tag=f"wk{k}", name=f"wk{k}") for k in range(KC)]
            wv_sb = [pp.tile([128, INNER], BF16, tag=f"wv{k}", name=f"wv{k}") for k in range(KC)]
            wo_sb = [pp.tile([128, D], BF16, tag=f"wo{m}", name=f"wo{m}") for m in range(2)]

            # ---- broadcast masks: bcast[p,f] = recip[head(p),f] via K=4 matmul
            emask_sb = pp.tile([4, 256], BF16, tag="emask", name="emask_sb")
            nc.sync.dma_start(emask_sb[:], emask[:])
            e_mask = [emask_sb[:, 128 * m:128 * (m + 1)] for m in range(2)]
            ident = pp.tile([128, 128], BF16, tag="ident", name="ident")
            make_identity(nc, ident[:])
            vpt_sb = [pp.tile([128, N], BF16, tag=f"vpt{m}", name=f"vpt{m}") for m in range(2)]

            # ---- input DMAs: k full tiles first, then q half tiles (chunk
            # ---- 0/1 halves first so the first exps start earliest)
            k_tiles = []
            q_half = {0: [], 1: []}
            for k in range(KC):
                t = xkv.tile([128, N], BF16, tag="xt", name="xkt")
                nc.sync.dma_start(t[:], kT[128 * k:128 * (k + 1), :])
                nc.gpsimd.dma_start(wk_sb[k][:], wk[128 * k:128 * (k + 1), :])
                k_tiles.append(t)
                tq = wk_pool.tile([128, N // 2], BF16, tag="es", name="xqt", bufs=ES_BUFS)
                nc.sync.dma_start(tq[:], qT[128 * k:128 * (k + 1), 0:1024])
                nc.gpsimd.dma_start(wq_sb[k][:], wq[128 * k:128 * (k + 1), :])
                q_half[0].append(tq)
            for k in range(KC):
                tq = wk_pool.tile([128, N // 2], BF16, tag="es", name="xqt", bufs=ES_BUFS)
                nc.sync.dma_start(tq[:], qT[128 * k:128 * (k + 1), 1024:2048])
                q_half[1].append(tq)

            # ---- projection emitters --------------------------------------
            qp_sb = [pp.tile([128, N], BF16, tag=f"qp{m}", name=f"qp{m}") for m in range(2)]
            kp_sb = [pp.tile([128, N], BF16, tag=f"kp{m}", name=f"kp{m}") for m in range(2)]
            vpa = [pp.tile([128, H_LOC * 65], BF16, tag=f"vpa{j}", name=f"vpa{j}") for j in range(KT)]

            def gen_xproj(w_sb, rhs_of, p_sb, m):
                """Generator: one projection (16 mms + copies per cc-group),
                yielding after each matmul so it can interleave with the
                softmax stream. PSUM on the 1-bank "epi" tag."""
                for cc in (0, 2):
                    ps2 = [
                        psum.tile([128, 512], F32, tag="epi", name="pps", bufs=2)
                        for _ in range(2)
                    ]
                    for k in range(KC):
                        for ci in range(2):
                            nc.tensor.matmul(
                                ps2[ci][:],
                                lhsT=w_sb[k][:, 128 * m:128 * (m + 1)],
                                rhs=rhs_of(k, cc + ci),
                                start=(k == 0),
                                stop=(k == KC - 1),
                            )
                            yield
                    for ci in range(2):
                        c = cc + ci
                        nc.vector.tensor_copy(
                            p_sb[m][:, 512 * c:512 * (c + 1)], ps2[ci][:]
                        )

            def gen_kproj(m):
                return gen_xproj(
                    wk_sb, lambda k, c: k_tiles[k][:, 512 * c:512 * (c + 1)],
                    kp_sb, m,
                )

            def gen_qproj(m):
                return gen_xproj(
                    wq_sb,
                    lambda k, c: q_half[c // 2][k][:, 512 * (c % 2):512 * (c % 2 + 1)],
                    qp_sb, m,
                )

            def gen_vproj(m):
                return gen_xproj(
                    wv_sb, lambda k, c: v_tiles[k][:, 512 * c:512 * (c + 1)],
                    vpt_sb, m,
                )

            slices = [(j, h) for j in range(KT) for h in range(2)]

            def emit_s_exp(m, c, b0):
                """One S+exp batch (2 ktile-slices, heads interleaved so the
                K=64 S-matmuls pack pairwise in the PE array)."""
                batch = slices[b0:b0 + 2]
                w = 512 * len(batch)
                sp = psum.tile([128, 1024], F32, tag="sp", name="sp", bufs=2)
                es = wk_pool.tile([128, 1024], BF16, tag="es", name="es", bufs=ES_BUFS)
                for s, (j, h) in enumerate(batch):
                    p0 = 64 * h
                    nc.tensor.matmul(
                        sp[:, 512 * s:512 * (s + 1)],
                        lhsT=kp_sb[m][p0:p0 + 64, 128 * j:128 * (j + 1)],
                        rhs=qp_sb[m][p0:p0 + 64, 512 * c:512 * (c + 1)],
                        start=True,
                        stop=True,
                    )
                nc.scalar.activation(
                    es[:, 0:w], sp[:, 0:w],
                    mybir.ActivationFunctionType.Exp, scale=SCALE,
                )
                return es

            def emit_o(m, b0, es, ot_ps):
                for s, (j, h) in enumerate(slices[b0:b0 + 2]):
                    hl = 2 * m + h
                    nc.tensor.matmul(
                        ot_ps[h][:],
                        lhsT=vpa[j][:, 65 * hl:65 * hl + 65],
                        rhs=es[:, 512 * s:512 * (s + 1)],
                        start=(j == 0),
                        stop=(j == KT - 1),
                    )

            def emit_vpa(j, m):
                tp = psum.tile([128, 128], BF16, tag="epi", name="tp", bufs=2)
                nc.tensor.transpose(
                    tp[:], vpt_sb[m][:, 128 * j:128 * (j + 1)], ident[:]
                )
                dst = vpa[j][:, 130 * m:130 * (m + 1)].rearrange(
                    "p (h e) -> p h e", e=65
                )[:, :, 0:64]
                nc.vector.tensor_copy(dst, tp[:].rearrange("p (h e) -> p h e", e=64))

            def new_ot_ps():
                return [
                    psum.tile([65, 512], F32, tag="otps", name=f"otps{h}", bufs=2)
                    for h in range(2)
                ]

            def unload_pair(m, ot_ps, pair_tile, den_c):
                # one 65-row copy per head (O rows + bf16 denominator row)
                # releases the PSUM accumulators after just two DVE ops
                stage_e = wk_pool.tile([65, 512], BF16, tag="stge", name="stge", bufs=2)
                stage_o = wk_pool.tile([65, 512], BF16, tag="stgo", name="stgo", bufs=2)
                nc.vector.tensor_copy(stage_e[:], ot_ps[0][:])
                nc.vector.tensor_copy(stage_o[:], ot_ps[1][:])
                nc.vector.tensor_copy(pair_tile[0:64, :], stage_e[0:64, :])
                nc.gpsimd.dma_start(pair_tile[64:128, :], stage_o[0:64, :])
                nc.gpsimd.dma_start(den_c[2 * m:2 * m + 1, :], stage_e[64:65, :])
                nc.gpsimd.dma_start(den_c[2 * m + 1:2 * m + 2, :], stage_o[64:65, :])

            def emit_epilogue(c, ot_sb, den_c):
                """normalize (recip -> mask-matmul broadcast -> multiply) and
                the Wo projection for one n_q chunk."""
                den_f = wk_pool.tile([4, 512], F32, tag="denf", name="denf", bufs=2)
                recip_f = wk_pool.tile([4, 512], F32, tag="recf", name="recf", bufs=2)
                recip_b = wk_pool.tile([4, 512], BF16, tag="recb", name="recb", bufs=2)
                nc.vector.tensor_copy(den_f[:], den_c[:])
                nc.vector.reciprocal_approx_fast(recip_f[:], den_f[:])
                nc.vector.tensor_copy(recip_b[:], recip_f[:])
                for m in range(2):
                    bc = psum.tile([128, 512], F32, tag="epi", name="bc", bufs=2)
                    nc.tensor.matmul(
                        bc[:], lhsT=e_mask[m], rhs=recip_b[:], start=True, stop=True,
                    )
                    nc.vector.tensor_mul(ot_sb[m][:], ot_sb[m][:], bc[:])
                for s in range(4):
                    for dch in range(2):
                        ops = psum.tile([128, 512], F32, tag="epi", name="op", bufs=2)
                        for m in range(2):
                            nc.tensor.matmul(
                                ops[:],
                                lhsT=ot_sb[m][:, 128 * s:128 * (s + 1)],
                                rhs=wo_sb[m][:, 512 * dch:512 * (dch + 1)],
                                start=(m == 0),
                                stop=(m == 1),
                            )
                        o_sb = wk_pool.tile([128, 512], F32, tag="osb", name="osb", bufs=2)
                        nc.vector.tensor_copy(o_sb[:], ops[:])
                        r0 = 512 * c + 128 * s
                        nc.sync.dma_start(
                            out[r0:r0 + 128, 512 * dch:512 * (dch + 1)], o_sb[:]
                        )

            # ---- emission schedule: 2-deep software pipeline ---------------
            # unit u = (pair m=u%2, chunk c=u//2). Unit u's S+exp batches
            # interleave with unit (u-2)'s O-pass; the prologue units carry
            # the pair-1 projections / V projection / PE transposes instead.
            # v inputs + remaining weights
            v_tiles = []
            for k in range(KC):
                t = xkv.tile([128, N], BF16, tag="xt", name="xvt")
                nc.sync.dma_start(t[:], vT[128 * k:128 * (k + 1), :])
                nc.gpsimd.dma_start(wv_sb[k][:], wv[128 * k:128 * (k + 1), :])
                v_tiles.append(t)
            for m in range(2):
                nc.gpsimd.dma_start(wo_sb[m][:], wo[128 * m:128 * (m + 1), :])

            for _ in gen_kproj(0):
                pass
            for _ in gen_qproj(0):
                pass

            units = [(u % 2, u // 2) for u in range(2 * NQC)]
            es_held = {}     # u -> list of es tiles awaiting the O pass
            ot_ps_of = {}
            ot_sb_of = {}
            den_of = {}

            def feeder_prologue0():
                # pair-1 K/Q projections: 64 matmuls
                for g in (gen_kproj(1), gen_qproj(1)):
                    for _ in g:
                        yield

            def feeder_prologue1():
                # transposed V projection: 64 matmuls
                for g in (gen_vproj(0), gen_vproj(1)):
                    for _ in g:
                        yield
                for j in range(KT):
                    nc.vector.memset(vpa[j][:], 1.0)

            for u, (m, c) in enumerate(units):
                # start-of-unit bookkeeping
                if u % 2 == 0:
                    den_of[c] = wk_pool.tile([4, 512], BF16, tag="den", name="den", bufs=2)
                ot_sb_of[u] = wk_pool.tile(
                    [128, 512], BF16, tag=f"ot{u}", name=f"ot{u}", bufs=1
                )
                if u == 0:
                    feeder = feeder_prologue0()
                elif u == 1:
                    feeder = feeder_prologue1()
                else:
                    feeder = None
                if u >= 2:
                    ot_ps_of[u - 2] = new_ot_ps()
                es_held[u] = []
                for b in range(KT):
                    if feeder is not None:
                        for _ in range(4):
                            next(feeder, None)
                    elif u >= 2:
                        up = u - 2
                        if up == 0:
                            emit_vpa(b, 0)
                            emit_vpa(b, 1)
                        emit_o(units[up][0], 2 * b, es_held[up][b], ot_ps_of[up])
                    es_held[u].append(emit_s_exp(m, c, 2 * b))
                if feeder is not None:
                    for _ in feeder:
                        pass
                if u >= 2:
                    up = u - 2
                    mp, cp = units[up]
                    unload_pair(mp, ot_ps_of[up], ot_sb_of[up], den_of[cp])
                    del es_held[up]
                    if mp == 1:
                        emit_epilogue(cp, [ot_sb_of[2 * cp], ot_sb_of[2 * cp + 1]], den_of[cp])

            # drain: O-passes for the last two units
            for up in (2 * NQC - 2, 2 * NQC - 1):
                mp, cp = units[up]
                ot_ps_of[up] = new_ot_ps()
                for b in range(KT):
                    emit_o(mp, 2 * b, es_held[up][b], ot_ps_of[up])
                unload_pair(mp, ot_ps_of[up], ot_sb_of[up], den_of[cp])
            emit_epilogue(NQC - 1, [ot_sb_of[2 * NQC - 2], ot_sb_of[2 * NQC - 1]], den_of[NQC - 1])

    nc.compile()
    return nc


_NC_CACHE = None


def _get_nc():
    global _NC_CACHE
    if _NC_CACHE is None:
        _NC_CACHE = _build_nc()
    return _NC_CACHE


def kernel(q, k, v, Wq, Wk, Wv, Wo):
    q = np.asarray(q, dtype=np.float32)
    k = np.asarray(k, dtype=np.float32)
    v = np.asarray(v, dtype=np.float32)
    Wq = np.asarray(Wq, dtype=np.float32)
    Wk = np.asarray(Wk, dtype=np.float32)
    Wv = np.asarray(Wv, dtype=np.float32)
    Wo = np.asarray(Wo, dtype=np.float32)

    qT = [np.ascontiguousarray(q[g].T).astype(NPBF16) for g in range(B)]
    kT = [np.ascontiguousarray(k[g].T).astype(NPBF16) for g in range(B)]
    vT = [np.ascontiguousarray(v[g].T).astype(NPBF16) for g in range(B)]
    wq_b = Wq.astype(NPBF16)
    wk_b = Wk.astype(NPBF16)
    wv_b = Wv.astype(NPBF16)
    wo_b = Wo.astype(NPBF16)
    emask = np.zeros((4, 256), NPBF16)
    for m in range(2):
        emask[2 * m, 128 * m:128 * m + 64] = 1
        emask[2 * m + 1, 128 * m + 64:128 * m + 128] = 1

    in_maps = []
    for c in range(8):
        g, t = c // 4, c % 4
        sl = slice(INNER * t, INNER * (t + 1))
        in_maps.append({
            "qT": qT[g],
            "kT": kT[g],
            "vT": vT[g],
            "wq": np.ascontiguousarray(wq_b[:, sl]),
            "wk": np.ascontiguousarray(wk_b[:, sl]),
            "wv": np.ascontiguousarray(wv_b[:, sl]),
            "wo": np.ascontiguousarray(wo_b[sl, :]),
            "emask": emask,
        })

    nc = _get_nc()
    res = run_bass_kernel_spmd(nc, in_maps, core_ids=list(range(8)))

    out = np.empty((B, N, D), np.float32)
    for g in range(B):
        acc = res.results[4 * g]["out"].astype(np.float32)
        for t in range(1, 4):
            acc = acc + res.results[4 * g + t]["out"]
        out[g] = acc
    return out


# revision 23
# speedup vs baseline: 1.0161x; 1.0161x over previous
"""Distributed Trainium2 Bass kernel for multi-head attention.

Problem: b=2, n=2048, dim=1024, heads=16, head_dim=64 (inner=1024), f32 I/O.

Sharding (Megatron-style, per the hint): data-parallel over batch (cores 0-3
handle batch 0, cores 4-7 batch 1) x tensor-parallel over heads (core c%4
owns heads 4*(c%4)..4*(c%4)+3 via column shards of Wq/Wk/Wv and row shards
of Wo). Each core produces a partial [n, dim] output (its 4 heads pushed
through its Wo row block); the unshard step sums the 4 partials per batch
(the "all-reduce after to_out" done at gather time -- measured on this fleet,
the on-device collective is ~60us/MB which would dominate the compute).

Per-core device pipeline (all matmuls bf16, f32 PSUM accumulation):
  1. qpT/kpT = Wq^T q^T etc in transposed [inner_loc, n] layout; vp in
     natural [n, inner_loc] layout padded with a ones column per head
     (so P@V also yields the softmax denominator for free as row 64).
  2. S^T = kh qh^T per head in [n_k, n_q] layout; exp on ScalarE with the
     1/sqrt(dh) scale folded into the activation; no max-subtraction
     (scores are ~N(0,1), exp is safe in f32).
  3. O^T (+denominator row) accumulated in PSUM over n_k tiles.
  4. Per n_q chunk: reciprocal of denominators, broadcast via a tiny
     mask-matmul, normalize O^T tiles, then the Wo projection emits the
     final [n_q, dim] rows in natural layout.

Scheduling: a dummy matmul burst warms the PE clock during the input DMAs;
K/Q projections for head-pair 0 run first so the chunk-0 softmax stream
starts on ScalarE as early as possible (its exp batches are deferred from
their P@V consumers to buy run-ahead); the V projection and the Wo
projection overlap the ScalarE-bound attention stream.
"""

import sys

if "/opt/trn_rl_repo" not in sys.path:
    sys.path.insert(0, "/opt/trn_rl_repo")

import numpy as np
import ml_dtypes

import concourse.bass as bass
import concourse.mybir as mybir
from concourse import bacc, tile
from concourse.bass_utils import run_bass_kernel_spmd
from concourse.masks import make_identity

BF16 = mybir.dt.bfloat16
F32 = mybir.dt.float32
NPBF16 = ml_dtypes.bfloat16

B = 2
N = 2048          # sequence length (full, per batch)
D = 1024          # model dim
H = 16            # total heads
DH = 64           # head dim
H_LOC = 4         # heads per core
INNER = H_LOC * DH  # 256, local inner dim
KC = D // 128     # 8 contraction chunks over model dim
KT = N // 128     # 16 k-tiles over sequence
NQC = N // 512    # 4 query chunks of 512
SCALE = DH ** -0.5
ES_BUFS = 36      # es slot pool (shared with the q input tiles)


def _build_nc():
    nc = bacc.Bacc("TRN2", target_bir_lowering=False, debug=False, num_devices=8)

    qT = nc.declare_dram_parameter("qT", [D, N], BF16, isOutput=False)
    kT = nc.declare_dram_parameter("kT", [D, N], BF16, isOutput=False)
    vT = nc.declare_dram_parameter("vT", [D, N], BF16, isOutput=False)
    wq = nc.declare_dram_parameter("wq", [D, INNER], BF16, isOutput=False)
    wk = nc.declare_dram_parameter("wk", [D, INNER], BF16, isOutput=False)
    wv = nc.declare_dram_parameter("wv", [D, INNER], BF16, isOutput=False)
    wo = nc.declare_dram_parameter("wo", [INNER, D], BF16, isOutput=False)
    emask = nc.declare_dram_parameter("emask", [4, 256], BF16, isOutput=False)
    out = nc.declare_dram_parameter("out", [N, D], F32, isOutput=True)

    with tile.TileContext(nc) as tc:
        with (
            tc.tile_pool(name="persist", bufs=1) as pp,
            tc.tile_pool(name="xkv", bufs=10) as xkv,
            tc.tile_pool(name="work", bufs=2) as wk_pool,
            tc.tile_pool(name="psum", bufs=2, space="PSUM") as psum,
        ):
            # ---- ScalarE exp table preload + PE clock warm-up burst
            warm = pp.tile([1, 16], F32, tag="warm", name="warm")
            nc.vector.memset(warm[:], 0.0)
            nc.scalar.activation(warm[:], warm[:], mybir.ActivationFunctionType.Exp)
            wa = pp.tile([128, 16], BF16, tag="wa", name="wa")
            wr = pp.tile([128, 512], BF16, tag="wr", name="wr")
            nc.vector.memset(wa[:], 0.0)
            nc.vector.memset(wr[:], 0.0)
            for i in range(10):
                wps = psum.tile([16, 512], F32, tag="epi", name="wps", bufs=2)
                nc.tensor.matmul(wps[:], lhsT=wa[:], rhs=wr[:], start=True, stop=True)

            # ---- persistent weight tiles
            wq_sb = [pp.tile([128, INNER], BF16, tag=f"wq{k}", name=f"wq{k}") for k in range(KC)]
            wk_sb = [pp.tile([128, INNER], BF16, tag=f"wk{k}", name=f"wk{k}") for k in range(KC)]
            wv_sb = [pp.tile([128, INNER], BF16, tag=f"wv{k}", name=f"wv{k}") for k in range(KC)]
            wo_sb = [pp.tile([128, D], BF16, tag=f"wo{m}", name=f"wo{m}") for m in range(2)]

            # ---- broadcast masks: bcast[p,f] = recip[head(p),f] via K=4 matmul
            emask_sb = pp.tile([4, 256], BF16, tag="emask", name="emask_sb")
            nc.sync.dma_start(emask_sb[:], emask[:])
            e_mask = [emask_sb[:, 128 * m:128 * (m + 1)] for m in range(2)]
            ident = pp.tile([128, 128], BF16, tag="ident", name="ident")
            make_identity(nc, ident[:])
            vpt_sb = [pp.tile([128, N], BF16, tag=f"vpt{m}", name=f"vpt{m}") for m in range(2)]

            # ---- input DMAs: k full tiles first, then q half tiles (chunk
            # ---- 0/1 halves first so the first exps start earliest)
            k_tiles = []
            for k in range(KC):
                t = xkv.tile([128, N], BF16, tag="xt", name="xkt")
                nc.sync.dma_start(t[:], kT[128 * k:128 * (k + 1), :])
                nc.gpsimd.dma_start(wk_sb[k][:], wk[128 * k:128 * (k + 1), :])
                k_tiles.append(t)
            q_half = {0: [], 1: []}
            for half in range(2):
                for k in range(KC):
                    t = wk_pool.tile([128, N // 2], BF16, tag="es", name="xqt", bufs=ES_BUFS)
                    nc.sync.dma_start(
                        t[:], qT[128 * k:128 * (k + 1), 1024 * half:1024 * (half + 1)]
                    )
                    if half == 0:
                        nc.gpsimd.dma_start(wq_sb[k][:], wq[128 * k:128 * (k + 1), :])
                    q_half[half].append(t)

            # ---- projection emitters --------------------------------------
            qp_sb = [pp.tile([128, N], BF16, tag=f"qp{m}", name=f"qp{m}") for m in range(2)]
            kp_sb = [pp.tile([128, N], BF16, tag=f"kp{m}", name=f"kp{m}") for m in range(2)]
            vpa = [pp.tile([128, H_LOC * 65], BF16, tag=f"vpa{j}", name=f"vpa{j}") for j in range(KT)]

            def gen_xproj(w_sb, rhs_of, p_sb, m):
                """Generator: one projection (16 mms + copies per cc-group),
                yielding after each matmul so it can interleave with the
                softmax stream. PSUM on the 1-bank "epi" tag."""
                for cc in (0, 2):
                    ps2 = [
                        psum.tile([128, 512], F32, tag="epi", name="pps", bufs=2)
                        for _ in range(2)
                    ]
                    for k in range(KC):
                        for ci in range(2):
                            nc.tensor.matmul(
                                ps2[ci][:],
                                lhsT=w_sb[k][:, 128 * m:128 * (m + 1)],
                                rhs=rhs_of(k, cc + ci),
                                start=(k == 0),
                                stop=(k == KC - 1),
                            )
                            yield
                    for ci in range(2):
                        c = cc + ci
                        nc.vector.tensor_copy(
                            p_sb[m][:, 512 * c:512 * (c + 1)], ps2[ci][:]
                        )

            def gen_kproj(m):
                return gen_xproj(
                    wk_sb, lambda k, c: k_tiles[k][:, 512 * c:512 * (c + 1)],
                    kp_sb, m,
                )

            def gen_qproj(m):
                return gen_xproj(
                    wq_sb,
                    lambda k, c: q_half[c // 2][k][:, 512 * (c % 2):512 * (c % 2 + 1)],
                    qp_sb, m,
                )

            def gen_vproj(m):
                return gen_xproj(
                    wv_sb, lambda k, c: v_tiles[k][:, 512 * c:512 * (c + 1)],
                    vpt_sb, m,
                )

            slices = [(j, h) for j in range(KT) for h in range(2)]

            def emit_s_exp(m, c, b0):
                """One S+exp batch (2 ktile-slices, heads interleaved so the
                K=64 S-matmuls pack pairwise in the PE array)."""
                batch = slices[b0:b0 + 2]
                w = 512 * len(batch)
                sp = psum.tile([128, 1024], F32, tag="sp", name="sp", bufs=2)
                es = wk_pool.tile([128, 1024], BF16, tag="es", name="es", bufs=ES_BUFS)
                for s, (j, h) in enumerate(batch):
                    p0 = 64 * h
                    nc.tensor.matmul(
                        sp[:, 512 * s:512 * (s + 1)],
                        lhsT=kp_sb[m][p0:p0 + 64, 128 * j:128 * (j + 1)],
                        rhs=qp_sb[m][p0:p0 + 64, 512 * c:512 * (c + 1)],
                        start=True,
                        stop=True,
                    )
                nc.scalar.activation(
                    es[:, 0:w], sp[:, 0:w],
                    mybir.ActivationFunctionType.Exp, scale=SCALE,
                )
                return es

            def emit_o(m, b0, es, ot_ps):
                for s, (j, h) in enumerate(slices[b0:b0 + 2]):
                    hl = 2 * m + h
                    nc.tensor.matmul(
                        ot_ps[h][:],
                        lhsT=vpa[j][:, 65 * hl:65 * hl + 65],
                        rhs=es[:, 512 * s:512 * (s + 1)],
                        start=(j == 0),
                        stop=(j == KT - 1),
                    )

            def emit_vpa(j, m):
                tp = psum.tile([128, 128], BF16, tag="epi", name="tp", bufs=2)
                nc.tensor.transpose(
                    tp[:], vpt_sb[m][:, 128 * j:128 * (j + 1)], ident[:]
                )
                dst = vpa[j][:, 130 * m:130 * (m + 1)].rearrange(
                    "p (h e) -> p h e", e=65
                )[:, :, 0:64]
                nc.vector.tensor_copy(dst, tp[:].rearrange("p (h e) -> p h e", e=64))

            def new_ot_ps():
                return [
                    psum.tile([65, 512], F32, tag="otps", name=f"otps{h}", bufs=2)
                    for h in range(2)
                ]

            def unload_pair(m, ot_ps, pair_tile, den_c):
                # one 65-row copy per head (O rows + bf16 denominator row)
                # releases the PSUM accumulators after just two DVE ops
                stage_e = wk_pool.tile([65, 512], BF16, tag="stge", name="stge", bufs=2)
                stage_o = wk_pool.tile([65, 512], BF16, tag="stgo", name="stgo", bufs=2)
                nc.vector.tensor_copy(stage_e[:], ot_ps[0][:])
                nc.vector.tensor_copy(stage_o[:], ot_ps[1][:])
                nc.vector.tensor_copy(pair_tile[0:64, :], stage_e[0:64, :])
                nc.sync.dma_start(pair_tile[64:128, :], stage_o[0:64, :])
                nc.sync.dma_start(den_c[2 * m:2 * m + 1, :], stage_e[64:65, :])
                nc.sync.dma_start(den_c[2 * m + 1:2 * m + 2, :], stage_o[64:65, :])

            def emit_epilogue(c, ot_sb, den_c):
                """normalize (recip -> mask-matmul broadcast -> multiply) and
                the Wo projection for one n_q chunk."""
                den_f = wk_pool.tile([4, 512], F32, tag="denf", name="denf", bufs=2)
                recip_f = wk_pool.tile([4, 512], F32, tag="recf", name="recf", bufs=2)
                recip_b = wk_pool.tile([4, 512], BF16, tag="recb", name="recb", bufs=2)
                nc.vector.tensor_copy(den_f[:], den_c[:])
                nc.vector.reciprocal_approx_fast(recip_f[:], den_f[:])
                nc.vector.tensor_copy(recip_b[:], recip_f[:])
                for m in range(2):
                    bc = psum.tile([128, 512], F32, tag="epi", name="bc", bufs=2)
                    nc.tensor.matmul(
                        bc[:], lhsT=e_mask[m], rhs=recip_b[:], start=True, stop=True,
                    )
                    nc.vector.tensor_mul(ot_sb[m][:], ot_sb[m][:], bc[:])
                for s in range(4):
                    for dch in range(2):
                        ops = psum.tile([128, 512], F32, tag="epi", name="op", bufs=2)
                        for m in range(2):
                            nc.tensor.matmul(
                                ops[:],
                                lhsT=ot_sb[m][:, 128 * s:128 * (s + 1)],
                                rhs=wo_sb[m][:, 512 * dch:512 * (dch + 1)],
                                start=(m == 0),
                                stop=(m == 1),
                            )
                        o_sb = wk_pool.tile([128, 512], F32, tag="osb", name="osb", bufs=2)
                        nc.vector.tensor_copy(o_sb[:], ops[:])
                        r0 = 512 * c + 128 * s
                        nc.sync.dma_start(
                            out[r0:r0 + 128, 512 * dch:512 * (dch + 1)], o_sb[:]
                        )

            # ---- emission schedule: 2-deep software pipeline ---------------
            # unit u = (pair m=u%2, chunk c=u//2). Unit u's S+exp batches
            # interleave with unit (u-2)'s O-pass; the prologue units carry
            # the pair-1 projections / V projection / PE transposes instead.
            # v inputs + remaining weights
            v_tiles = []
            for k in range(KC):
                t = xkv.tile([128, N], BF16, tag="xt", name="xvt")
                nc.sync.dma_start(t[:], vT[128 * k:128 * (k + 1), :])
                nc.gpsimd.dma_start(wv_sb[k][:], wv[128 * k:128 * (k + 1), :])
                v_tiles.append(t)
            for m in range(2):
                nc.gpsimd.dma_start(wo_sb[m][:], wo[128 * m:128 * (m + 1), :])

            for _ in gen_kproj(0):
                pass
            for _ in gen_qproj(0):
                pass

            units = [(u % 2, u // 2) for u in range(2 * NQC)]
            es_held = {}     # u -> list of es tiles awaiting the O pass
            ot_ps_of = {}
            ot_sb_of = {}
            den_of = {}

            def feeder_prologue0():
                # pair-1 K/Q projections: 64 matmuls
                for g in (gen_kproj(1), gen_qproj(1)):
                    for _ in g:
                        yield

            def feeder_prologue1():
                # transposed V projection: 64 matmuls
                for g in (gen_vproj(0), gen_vproj(1)):
                    for _ in g:
                        yield
                for j in range(KT):
                    nc.vector.memset(vpa[j][:], 1.0)

            for u, (m, c) in enumerate(units):
                # start-of-unit bookkeeping
                if u % 2 == 0:
                    den_of[c] = wk_pool.tile([4, 512], BF16, tag="den", name="den", bufs=2)
                ot_sb_of[u] = wk_pool.tile(
                    [128, 512], BF16, tag=f"ot{u}", name=f"ot{u}", bufs=1
                )
                if u == 0:
                    feeder = feeder_prologue0()
                elif u == 1:
                    feeder = feeder_prologue1()
                else:
                    feeder = None
                if u >= 2:
                    ot_ps_of[u - 2] = new_ot_ps()
                es_held[u] = []
                for b in range(KT):
                    if feeder is not None:
                        for _ in range(4):
                            next(feeder, None)
                    elif u >= 2:
                        up = u - 2
                        if up == 0:
                            emit_vpa(b, 0)
                            emit_vpa(b, 1)
                        emit_o(units[up][0], 2 * b, es_held[up][b], ot_ps_of[up])
                    es_held[u].append(emit_s_exp(m, c, 2 * b))
                if feeder is not None:
                    for _ in feeder:
                        pass
                if u >= 2:
                    up = u - 2
                    mp, cp = units[up]
                    unload_pair(mp, ot_ps_of[up], ot_sb_of[up], den_of[cp])
                    del es_held[up]
                    if mp == 1:
                        emit_epilogue(cp, [ot_sb_of[2 * cp], ot_sb_of[2 * cp + 1]], den_of[cp])

            # drain: O-passes for the last two units
            for up in (2 * NQC - 2, 2 * NQC - 1):
                mp, cp = units[up]
                ot_ps_of[up] = new_ot_ps()
                for b in range(KT):
                    emit_o(mp, 2 * b, es_held[up][b], ot_ps_of[up])
                unload_pair(mp, ot_ps_of[up], ot_sb_of[up], den_of[cp])
            emit_epilogue(NQC - 1, [ot_sb_of[2 * NQC - 2], ot_sb_of[2 * NQC - 1]], den_of[NQC - 1])

    nc.compile()
    return nc


_NC_CACHE = None


def _get_nc():
    global _NC_CACHE
    if _NC_CACHE is None:
        _NC_CACHE = _build_nc()
    return _NC_CACHE


def kernel(q, k, v, Wq, Wk, Wv, Wo):
    q = np.asarray(q, dtype=np.float32)
    k = np.asarray(k, dtype=np.float32)
    v = np.asarray(v, dtype=np.float32)
    Wq = np.asarray(Wq, dtype=np.float32)
    Wk = np.asarray(Wk, dtype=np.float32)
    Wv = np.asarray(Wv, dtype=np.float32)
    Wo = np.asarray(Wo, dtype=np.float32)

    qT = [np.ascontiguousarray(q[g].T).astype(NPBF16) for g in range(B)]
    kT = [np.ascontiguousarray(k[g].T).astype(NPBF16) for g in range(B)]
    vT = [np.ascontiguousarray(v[g].T).astype(NPBF16) for g in range(B)]
    wq_b = Wq.astype(NPBF16)
    wk_b = Wk.astype(NPBF16)
    wv_b = Wv.astype(NPBF16)
    wo_b = Wo.astype(NPBF16)
    emask = np.zeros((4, 256), NPBF16)
    for m in range(2):
        emask[2 * m, 128 * m:128 * m + 64] = 1
        emask[2 * m + 1, 128 * m + 64:128 * m + 128] = 1

    in_maps = []
    for c in range(8):
        g, t = c // 4, c % 4
        sl = slice(INNER * t, INNER * (t + 1))
        in_maps.append({
            "qT": qT[g],
            "kT": kT[g],
            "vT": vT[g],
            "wq": np.ascontiguousarray(wq_b[:, sl]),
            "wk": np.ascontiguousarray(wk_b[:, sl]),
            "wv": np.ascontiguousarray(wv_b[:, sl]),
            "wo": np.ascontiguousarray(wo_b[sl, :]),
            "emask": emask,
        })

    nc = _get_nc()
    res = run_bass_kernel_spmd(nc, in_maps, core_ids=list(range(8)))

    out = np.empty((B, N, D), np.float32)
    for g in range(B):
        acc = res.results[4 * g]["out"].astype(np.float32)
        for t in range(1, 4):
            acc = acc + res.results[4 * g + t]["out"]
        out[g] = acc
    return out


# revision 24
# speedup vs baseline: 1.0170x; 1.0009x over previous
"""Distributed Trainium2 Bass kernel for multi-head attention.

Problem: b=2, n=2048, dim=1024, heads=16, head_dim=64 (inner=1024), f32 I/O.

Sharding (Megatron-style, per the hint): data-parallel over batch (cores 0-3
handle batch 0, cores 4-7 batch 1) x tensor-parallel over heads (core c%4
owns heads 4*(c%4)..4*(c%4)+3 via column shards of Wq/Wk/Wv and row shards
of Wo). Each core produces a partial [n, dim] output (its 4 heads pushed
through its Wo row block); the unshard step sums the 4 partials per batch
(the "all-reduce after to_out" done at gather time -- measured on this fleet,
the on-device collective is ~60us/MB which would dominate the compute).

Per-core device pipeline (all matmuls bf16, f32 PSUM accumulation):
  1. qpT/kpT = Wq^T q^T etc in transposed [inner_loc, n] layout; vp in
     natural [n, inner_loc] layout padded with a ones column per head
     (so P@V also yields the softmax denominator for free as row 64).
  2. S^T = kh qh^T per head in [n_k, n_q] layout; exp on ScalarE with the
     1/sqrt(dh) scale folded into the activation; no max-subtraction
     (scores are ~N(0,1), exp is safe in f32).
  3. O^T (+denominator row) accumulated in PSUM over n_k tiles.
  4. Per n_q chunk: reciprocal of denominators, broadcast via a tiny
     mask-matmul, normalize O^T tiles, then the Wo projection emits the
     final [n_q, dim] rows in natural layout.

Scheduling: a dummy matmul burst warms the PE clock during the input DMAs;
K/Q projections for head-pair 0 run first so the chunk-0 softmax stream
starts on ScalarE as early as possible (its exp batches are deferred from
their P@V consumers to buy run-ahead); the V projection and the Wo
projection overlap the ScalarE-bound attention stream.
"""

import sys

if "/opt/trn_rl_repo" not in sys.path:
    sys.path.insert(0, "/opt/trn_rl_repo")

import numpy as np
import ml_dtypes

import concourse.bass as bass
import concourse.mybir as mybir
from concourse import bacc, tile
from concourse.bass_utils import run_bass_kernel_spmd
from concourse.masks import make_identity

BF16 = mybir.dt.bfloat16
F32 = mybir.dt.float32
NPBF16 = ml_dtypes.bfloat16

B = 2
N = 2048          # sequence length (full, per batch)
D = 1024          # model dim
H = 16            # total heads
DH = 64           # head dim
H_LOC = 4         # heads per core
INNER = H_LOC * DH  # 256, local inner dim
KC = D // 128     # 8 contraction chunks over model dim
KT = N // 128     # 16 k-tiles over sequence
NQC = N // 512    # 4 query chunks of 512
SCALE = DH ** -0.5
ES_BUFS = 36      # es slot pool (shared with the q input tiles)


def _build_nc():
    nc = bacc.Bacc("TRN2", target_bir_lowering=False, debug=False, num_devices=8)

    qT = nc.declare_dram_parameter("qT", [D, N], BF16, isOutput=False)
    kT = nc.declare_dram_parameter("kT", [D, N], BF16, isOutput=False)
    vT = nc.declare_dram_parameter("vT", [D, N], BF16, isOutput=False)
    wq = nc.declare_dram_parameter("wq", [D, INNER], BF16, isOutput=False)
    wk = nc.declare_dram_parameter("wk", [D, INNER], BF16, isOutput=False)
    wv = nc.declare_dram_parameter("wv", [D, INNER], BF16, isOutput=False)
    wo = nc.declare_dram_parameter("wo", [INNER, D], BF16, isOutput=False)
    emask = nc.declare_dram_parameter("emask", [4, 256], BF16, isOutput=False)
    out = nc.declare_dram_parameter("out", [N, D], F32, isOutput=True)

    with tile.TileContext(nc) as tc:
        with (
            tc.tile_pool(name="persist", bufs=1) as pp,
            tc.tile_pool(name="xkv", bufs=10) as xkv,
            tc.tile_pool(name="work", bufs=2) as wk_pool,
            tc.tile_pool(name="psum", bufs=2, space="PSUM") as psum,
        ):
            # ---- ScalarE exp table preload + PE clock warm-up burst
            warm = pp.tile([1, 16], F32, tag="warm", name="warm")
            nc.vector.memset(warm[:], 0.0)
            nc.scalar.activation(warm[:], warm[:], mybir.ActivationFunctionType.Exp)
            wa = pp.tile([128, 16], BF16, tag="wa", name="wa")
            wr = pp.tile([128, 512], BF16, tag="wr", name="wr")
            nc.vector.memset(wa[:], 0.0)
            nc.vector.memset(wr[:], 0.0)
            for i in range(10):
                wps = psum.tile([16, 512], F32, tag="epi", name="wps", bufs=2)
                nc.tensor.matmul(wps[:], lhsT=wa[:], rhs=wr[:], start=True, stop=True)

            # ---- persistent weight tiles
            wq_sb = [pp.tile([128, INNER], BF16, tag=f"wq{k}", name=f"wq{k}") for k in range(KC)]
            wk_sb = [pp.tile([128, INNER], BF16, tag=f"wk{k}", name=f"wk{k}") for k in range(KC)]
            wv_sb = [pp.tile([128, INNER], BF16, tag=f"wv{k}", name=f"wv{k}") for k in range(KC)]
            wo_sb = [pp.tile([128, D], BF16, tag=f"wo{m}", name=f"wo{m}") for m in range(2)]

            # ---- broadcast masks: bcast[p,f] = recip[head(p),f] via K=4 matmul
            emask_sb = pp.tile([4, 256], BF16, tag="emask", name="emask_sb")
            nc.sync.dma_start(emask_sb[:], emask[:])
            e_mask = [emask_sb[:, 128 * m:128 * (m + 1)] for m in range(2)]
            ident = pp.tile([128, 128], BF16, tag="ident", name="ident")
            make_identity(nc, ident[:])
            vpt_sb = [pp.tile([128, N], BF16, tag=f"vpt{m}", name=f"vpt{m}") for m in range(2)]

            # ---- input DMAs: k full tiles first, then q half tiles (chunk
            # ---- 0/1 halves first so the first exps start earliest)
            k_tiles = []
            for k in range(KC):
                t = xkv.tile([128, N], BF16, tag="xt", name="xkt")
                nc.sync.dma_start(t[:], kT[128 * k:128 * (k + 1), :])
                nc.gpsimd.dma_start(wk_sb[k][:], wk[128 * k:128 * (k + 1), :])
                k_tiles.append(t)
            q_half = {0: [], 1: []}
            for half in range(2):
                for k in range(KC):
                    t = wk_pool.tile([128, N // 2], BF16, tag="es", name="xqt", bufs=ES_BUFS)
                    nc.sync.dma_start(
                        t[:], qT[128 * k:128 * (k + 1), 1024 * half:1024 * (half + 1)]
                    )
                    if half == 0:
                        nc.gpsimd.dma_start(wq_sb[k][:], wq[128 * k:128 * (k + 1), :])
                    q_half[half].append(t)

            # ---- projection emitters --------------------------------------
            qp_sb = [pp.tile([128, N], BF16, tag=f"qp{m}", name=f"qp{m}") for m in range(2)]
            kp_sb = [pp.tile([128, N], BF16, tag=f"kp{m}", name=f"kp{m}") for m in range(2)]
            vpa = [pp.tile([128, H_LOC * 65], BF16, tag=f"vpa{j}", name=f"vpa{j}") for j in range(KT)]

            def gen_xproj(w_sb, rhs_of, p_sb, m):
                """Generator: one projection (16 mms + copies per cc-group),
                yielding after each matmul so it can interleave with the
                softmax stream. PSUM on the 1-bank "epi" tag."""
                for cc in (0, 2):
                    ps2 = [
                        psum.tile([128, 512], F32, tag="epi", name="pps", bufs=2)
                        for _ in range(2)
                    ]
                    for k in range(KC):
                        for ci in range(2):
                            nc.tensor.matmul(
                                ps2[ci][:],
                                lhsT=w_sb[k][:, 128 * m:128 * (m + 1)],
                                rhs=rhs_of(k, cc + ci),
                                start=(k == 0),
                                stop=(k == KC - 1),
                            )
                            yield
                    for ci in range(2):
                        c = cc + ci
                        nc.vector.tensor_copy(
                            p_sb[m][:, 512 * c:512 * (c + 1)], ps2[ci][:]
                        )

            def gen_kproj(m):
                return gen_xproj(
                    wk_sb, lambda k, c: k_tiles[k][:, 512 * c:512 * (c + 1)],
                    kp_sb, m,
                )

            def gen_qproj(m):
                return gen_xproj(
                    wq_sb,
                    lambda k, c: q_half[c // 2][k][:, 512 * (c % 2):512 * (c % 2 + 1)],
                    qp_sb, m,
                )

            def gen_vproj(m):
                return gen_xproj(
                    wv_sb, lambda k, c: v_tiles[k][:, 512 * c:512 * (c + 1)],
                    vpt_sb, m,
                )

            slices = [(j, h) for j in range(KT) for h in range(2)]

            def emit_s_exp(m, c, b0):
                """One S+exp batch (2 ktile-slices, heads interleaved so the
                K=64 S-matmuls pack pairwise in the PE array)."""
                batch = slices[b0:b0 + 2]
                w = 512 * len(batch)
                sp = psum.tile([128, 1024], F32, tag="sp", name="sp", bufs=2)
                es = wk_pool.tile([128, 1024], BF16, tag="es", name="es", bufs=ES_BUFS)
                for s, (j, h) in enumerate(batch):
                    p0 = 64 * h
                    nc.tensor.matmul(
                        sp[:, 512 * s:512 * (s + 1)],
                        lhsT=kp_sb[m][p0:p0 + 64, 128 * j:128 * (j + 1)],
                        rhs=qp_sb[m][p0:p0 + 64, 512 * c:512 * (c + 1)],
                        start=True,
                        stop=True,
                    )
                nc.scalar.activation(
                    es[:, 0:w], sp[:, 0:w],
                    mybir.ActivationFunctionType.Exp, scale=SCALE,
                )
                return es

            def emit_o(m, b0, es, ot_ps):
                for s, (j, h) in enumerate(slices[b0:b0 + 2]):
                    hl = 2 * m + h
                    nc.tensor.matmul(
                        ot_ps[h][:],
                        lhsT=vpa[j][:, 65 * hl:65 * hl + 65],
                        rhs=es[:, 512 * s:512 * (s + 1)],
                        start=(j == 0),
                        stop=(j == KT - 1),
                    )

            def emit_vpa(j, m):
                tp = psum.tile([128, 128], BF16, tag="epi", name="tp", bufs=2)
                nc.tensor.transpose(
                    tp[:], vpt_sb[m][:, 128 * j:128 * (j + 1)], ident[:]
                )
                dst = vpa[j][:, 130 * m:130 * (m + 1)].rearrange(
                    "p (h e) -> p h e", e=65
                )[:, :, 0:64]
                nc.vector.tensor_copy(dst, tp[:].rearrange("p (h e) -> p h e", e=64))

            def new_ot_ps():
                return [
                    psum.tile([65, 512], F32, tag="otps", name=f"otps{h}", bufs=2)
                    for h in range(2)
                ]

            def unload_pair(m, ot_ps, pair_tile, den_c, tail=False):
                # one 65-row copy per head (O rows + bf16 denominator row)
                # releases the PSUM accumulators after just two ops; in the
                # tail these ride the otherwise-idle ScalarE
                stage_e = wk_pool.tile([65, 512], BF16, tag="stge", name="stge", bufs=2)
                stage_o = wk_pool.tile([65, 512], BF16, tag="stgo", name="stgo", bufs=2)
                eng = nc.scalar if tail else nc.vector
                if tail:
                    eng.copy(stage_e[:], ot_ps[0][:])
                    eng.copy(stage_o[:], ot_ps[1][:])
                else:
                    nc.vector.tensor_copy(stage_e[:], ot_ps[0][:])
                    nc.vector.tensor_copy(stage_o[:], ot_ps[1][:])
                nc.vector.tensor_copy(pair_tile[0:64, :], stage_e[0:64, :])
                nc.sync.dma_start(pair_tile[64:128, :], stage_o[0:64, :])
                nc.sync.dma_start(den_c[2 * m:2 * m + 1, :], stage_e[64:65, :])
                nc.sync.dma_start(den_c[2 * m + 1:2 * m + 2, :], stage_o[64:65, :])

            def emit_epilogue(c, ot_sb, den_c, tail=False):
                """normalize (recip -> mask-matmul broadcast -> multiply) and
                the Wo projection for one n_q chunk."""
                den_f = wk_pool.tile([4, 512], F32, tag="denf", name="denf", bufs=2)
                recip_f = wk_pool.tile([4, 512], F32, tag="recf", name="recf", bufs=2)
                recip_b = wk_pool.tile([4, 512], BF16, tag="recb", name="recb", bufs=2)
                nc.vector.tensor_copy(den_f[:], den_c[:])
                nc.vector.reciprocal_approx_fast(recip_f[:], den_f[:])
                nc.vector.tensor_copy(recip_b[:], recip_f[:])
                for m in range(2):
                    bc = psum.tile([128, 512], F32, tag="epi", name="bc", bufs=2)
                    nc.tensor.matmul(
                        bc[:], lhsT=e_mask[m], rhs=recip_b[:], start=True, stop=True,
                    )
                    nc.vector.tensor_mul(ot_sb[m][:], ot_sb[m][:], bc[:])
                for s in range(4):
                    for dch in range(2):
                        ops = psum.tile([128, 512], F32, tag="epi", name="op", bufs=2)
                        for m in range(2):
                            nc.tensor.matmul(
                                ops[:],
                                lhsT=ot_sb[m][:, 128 * s:128 * (s + 1)],
                                rhs=wo_sb[m][:, 512 * dch:512 * (dch + 1)],
                                start=(m == 0),
                                stop=(m == 1),
                            )
                        o_sb = wk_pool.tile([128, 512], F32, tag="osb", name="osb", bufs=2)
                        if tail:
                            nc.scalar.copy(o_sb[:], ops[:])
                        else:
                            nc.vector.tensor_copy(o_sb[:], ops[:])
                        r0 = 512 * c + 128 * s
                        nc.sync.dma_start(
                            out[r0:r0 + 128, 512 * dch:512 * (dch + 1)], o_sb[:]
                        )

            # ---- emission schedule: 2-deep software pipeline ---------------
            # unit u = (pair m=u%2, chunk c=u//2). Unit u's S+exp batches
            # interleave with unit (u-2)'s O-pass; the prologue units carry
            # the pair-1 projections / V projection / PE transposes instead.
            # v inputs + remaining weights
            v_tiles = []
            for k in range(KC):
                t = xkv.tile([128, N], BF16, tag="xt", name="xvt")
                nc.sync.dma_start(t[:], vT[128 * k:128 * (k + 1), :])
                nc.gpsimd.dma_start(wv_sb[k][:], wv[128 * k:128 * (k + 1), :])
                v_tiles.append(t)
            for m in range(2):
                nc.gpsimd.dma_start(wo_sb[m][:], wo[128 * m:128 * (m + 1), :])

            for _ in gen_kproj(0):
                pass
            for _ in gen_qproj(0):
                pass

            units = [(u % 2, u // 2) for u in range(2 * NQC)]
            es_held = {}     # u -> list of es tiles awaiting the O pass
            ot_ps_of = {}
            ot_sb_of = {}
            den_of = {}

            def feeder_prologue0():
                # pair-1 K/Q projections: 64 matmuls
                for g in (gen_kproj(1), gen_qproj(1)):
                    for _ in g:
                        yield

            def feeder_prologue1():
                # transposed V projection: 64 matmuls
                for g in (gen_vproj(0), gen_vproj(1)):
                    for _ in g:
                        yield
                for j in range(KT):
                    nc.vector.memset(vpa[j][:], 1.0)

            for u, (m, c) in enumerate(units):
                # start-of-unit bookkeeping
                if u % 2 == 0:
                    den_of[c] = wk_pool.tile([4, 512], BF16, tag="den", name="den", bufs=2)
                ot_sb_of[u] = wk_pool.tile(
                    [128, 512], BF16, tag=f"ot{u}", name=f"ot{u}", bufs=1
                )
                if u == 0:
                    feeder = feeder_prologue0()
                elif u == 1:
                    feeder = feeder_prologue1()
                else:
                    feeder = None
                if u >= 2:
                    ot_ps_of[u - 2] = new_ot_ps()
                es_held[u] = []
                for b in range(KT):
                    if feeder is not None:
                        for _ in range(4):
                            next(feeder, None)
                    elif u >= 2:
                        up = u - 2
                        if up == 0:
                            emit_vpa(b, 0)
                            emit_vpa(b, 1)
                        emit_o(units[up][0], 2 * b, es_held[up][b], ot_ps_of[up])
                    es_held[u].append(emit_s_exp(m, c, 2 * b))
                if feeder is not None:
                    for _ in feeder:
                        pass
                if u >= 2:
                    up = u - 2
                    mp, cp = units[up]
                    unload_pair(mp, ot_ps_of[up], ot_sb_of[up], den_of[cp])
                    del es_held[up]
                    if mp == 1:
                        emit_epilogue(
                            cp, [ot_sb_of[2 * cp], ot_sb_of[2 * cp + 1]],
                            den_of[cp], tail=(cp >= 2),
                        )

            # drain: O-passes for the last two units
            for up in (2 * NQC - 2, 2 * NQC - 1):
                mp, cp = units[up]
                ot_ps_of[up] = new_ot_ps()
                for b in range(KT):
                    emit_o(mp, 2 * b, es_held[up][b], ot_ps_of[up])
                unload_pair(mp, ot_ps_of[up], ot_sb_of[up], den_of[cp], tail=True)
            emit_epilogue(
                NQC - 1, [ot_sb_of[2 * NQC - 2], ot_sb_of[2 * NQC - 1]],
                den_of[NQC - 1], tail=True,
            )

    nc.compile()
    return nc


_NC_CACHE = None


def _get_nc():
    global _NC_CACHE
    if _NC_CACHE is None:
        _NC_CACHE = _build_nc()
    return _NC_CACHE


def kernel(q, k, v, Wq, Wk, Wv, Wo):
    q = np.asarray(q, dtype=np.float32)
    k = np.asarray(k, dtype=np.float32)
    v = np.asarray(v, dtype=np.float32)
    Wq = np.asarray(Wq, dtype=np.float32)
    Wk = np.asarray(Wk, dtype=np.float32)
    Wv = np.asarray(Wv, dtype=np.float32)
    Wo = np.asarray(Wo, dtype=np.float32)

    qT = [np.ascontiguousarray(q[g].T).astype(NPBF16) for g in range(B)]
    kT = [np.ascontiguousarray(k[g].T).astype(NPBF16) for g in range(B)]
    vT = [np.ascontiguousarray(v[g].T).astype(NPBF16) for g in range(B)]
    wq_b = Wq.astype(NPBF16)
    wk_b = Wk.astype(NPBF16)
    wv_b = Wv.astype(NPBF16)
    wo_b = Wo.astype(NPBF16)
    emask = np.zeros((4, 256), NPBF16)
    for m in range(2):
        emask[2 * m, 128 * m:128 * m + 64] = 1
        emask[2 * m + 1, 128 * m + 64:128 * m + 128] = 1

    in_maps = []
    for c in range(8):
        g, t = c // 4, c % 4
        sl = slice(INNER * t, INNER * (t + 1))
        in_maps.append({
            "qT": qT[g],
            "kT": kT[g],
            "vT": vT[g],
            "wq": np.ascontiguousarray(wq_b[:, sl]),
            "wk": np.ascontiguousarray(wk_b[:, sl]),
            "wv": np.ascontiguousarray(wv_b[:, sl]),
            "wo": np.ascontiguousarray(wo_b[sl, :]),
            "emask": emask,
        })

    nc = _get_nc()
    res = run_bass_kernel_spmd(nc, in_maps, core_ids=list(range(8)))

    out = np.empty((B, N, D), np.float32)
    for g in range(B):
        acc = res.results[4 * g]["out"].astype(np.float32)
        for t in range(1, 4):
            acc = acc + res.results[4 * g + t]["out"]
        out[g] = acc
    return out


# revision 25
# speedup vs baseline: 1.0184x; 1.0014x over previous
"""Distributed Trainium2 Bass kernel for multi-head attention.

Problem: b=2, n=2048, dim=1024, heads=16, head_dim=64 (inner=1024), f32 I/O.

Sharding (Megatron-style, per the hint): data-parallel over batch (cores 0-3
handle batch 0, cores 4-7 batch 1) x tensor-parallel over heads (core c%4
owns heads 4*(c%4)..4*(c%4)+3 via column shards of Wq/Wk/Wv and row shards
of Wo). Each core produces a partial [n, dim] output (its 4 heads pushed
through its Wo row block); the unshard step sums the 4 partials per batch
(the "all-reduce after to_out" done at gather time -- measured on this fleet,
the on-device collective is ~60us/MB which would dominate the compute).

Per-core device pipeline (all matmuls bf16, f32 PSUM accumulation):
  1. qpT/kpT = Wq^T q^T etc in transposed [inner_loc, n] layout; vp in
     natural [n, inner_loc] layout padded with a ones column per head
     (so P@V also yields the softmax denominator for free as row 64).
  2. S^T = kh qh^T per head in [n_k, n_q] layout; exp on ScalarE with the
     1/sqrt(dh) scale folded into the activation; no max-subtraction
     (scores are ~N(0,1), exp is safe in f32).
  3. O^T (+denominator row) accumulated in PSUM over n_k tiles.
  4. Per n_q chunk: reciprocal of denominators, broadcast via a tiny
     mask-matmul, normalize O^T tiles, then the Wo projection emits the
     final [n_q, dim] rows in natural layout.

Scheduling: a dummy matmul burst warms the PE clock during the input DMAs;
K/Q projections for head-pair 0 run first so the chunk-0 softmax stream
starts on ScalarE as early as possible (its exp batches are deferred from
their P@V consumers to buy run-ahead); the V projection and the Wo
projection overlap the ScalarE-bound attention stream.
"""

import sys

if "/opt/trn_rl_repo" not in sys.path:
    sys.path.insert(0, "/opt/trn_rl_repo")

import numpy as np
import ml_dtypes

import concourse.bass as bass
import concourse.mybir as mybir
from concourse import bacc, tile
from concourse.bass_utils import run_bass_kernel_spmd
from concourse.masks import make_identity

BF16 = mybir.dt.bfloat16
F32 = mybir.dt.float32
NPBF16 = ml_dtypes.bfloat16

B = 2
N = 2048          # sequence length (full, per batch)
D = 1024          # model dim
H = 16            # total heads
DH = 64           # head dim
H_LOC = 4         # heads per core
INNER = H_LOC * DH  # 256, local inner dim
KC = D // 128     # 8 contraction chunks over model dim
KT = N // 128     # 16 k-tiles over sequence
NQC = N // 512    # 4 query chunks of 512
SCALE = DH ** -0.5
ES_BUFS = 36      # es slot pool (shared with the q input tiles)


def _build_nc():
    nc = bacc.Bacc("TRN2", target_bir_lowering=False, debug=False, num_devices=8)

    qT = nc.declare_dram_parameter("qT", [D, N], BF16, isOutput=False)
    kT = nc.declare_dram_parameter("kT", [D, N], BF16, isOutput=False)
    vT = nc.declare_dram_parameter("vT", [D, N], BF16, isOutput=False)
    wq = nc.declare_dram_parameter("wq", [D, INNER], BF16, isOutput=False)
    wk = nc.declare_dram_parameter("wk", [D, INNER], BF16, isOutput=False)
    wv = nc.declare_dram_parameter("wv", [D, INNER], BF16, isOutput=False)
    wo = nc.declare_dram_parameter("wo", [INNER, D], BF16, isOutput=False)
    emask = nc.declare_dram_parameter("emask", [4, 256], BF16, isOutput=False)
    out = nc.declare_dram_parameter("out", [N, D], F32, isOutput=True)

    with tile.TileContext(nc) as tc:
        with (
            tc.tile_pool(name="persist", bufs=1) as pp,
            tc.tile_pool(name="xkv", bufs=10) as xkv,
            tc.tile_pool(name="work", bufs=2) as wk_pool,
            tc.tile_pool(name="psum", bufs=2, space="PSUM") as psum,
        ):
            # ---- ScalarE exp table preload + PE clock warm-up burst
            warm = pp.tile([1, 16], F32, tag="warm", name="warm")
            nc.vector.memset(warm[:], 0.0)
            nc.scalar.activation(warm[:], warm[:], mybir.ActivationFunctionType.Exp)
            wa = pp.tile([128, 16], BF16, tag="wa", name="wa")
            wr = pp.tile([128, 512], BF16, tag="wr", name="wr")
            nc.vector.memset(wa[:], 0.0)
            nc.vector.memset(wr[:], 0.0)
            for i in range(10):
                wps = psum.tile([16, 512], F32, tag="epi", name="wps", bufs=2)
                nc.tensor.matmul(wps[:], lhsT=wa[:], rhs=wr[:], start=True, stop=True)

            # ---- persistent weight tiles
            wq_sb = [pp.tile([128, INNER], BF16, tag=f"wq{k}", name=f"wq{k}") for k in range(KC)]
            wk_sb = [pp.tile([128, INNER], BF16, tag=f"wk{k}", name=f"wk{k}") for k in range(KC)]
            wv_sb = [pp.tile([128, INNER], BF16, tag=f"wv{k}", name=f"wv{k}") for k in range(KC)]
            wo_sb = [pp.tile([128, D], BF16, tag=f"wo{m}", name=f"wo{m}") for m in range(2)]

            # ---- broadcast masks: bcast[p,f] = recip[head(p),f] via K=4 matmul
            emask_sb = pp.tile([4, 256], BF16, tag="emask", name="emask_sb")
            nc.sync.dma_start(emask_sb[:], emask[:])
            e_mask = [emask_sb[:, 128 * m:128 * (m + 1)] for m in range(2)]
            ident = pp.tile([128, 128], BF16, tag="ident", name="ident")
            make_identity(nc, ident[:])
            vpt_sb = [pp.tile([128, N], BF16, tag=f"vpt{m}", name=f"vpt{m}") for m in range(2)]

            # ---- input DMAs: k full tiles first, then q half tiles (chunk
            # ---- 0/1 halves first so the first exps start earliest)
            # k tiles stream in sequence-halves: the first kproj group only
            # reads columns 0-1023, so it unblocks after half the k bytes
            k_tiles = []
            for k in range(KC):
                t = xkv.tile([128, N], BF16, tag="xt", name="xkt")
                nc.sync.dma_start(t[:, 0:1024], kT[128 * k:128 * (k + 1), 0:1024])
                nc.gpsimd.dma_start(wk_sb[k][:], wk[128 * k:128 * (k + 1), :])
                k_tiles.append(t)
            q_half = {0: [], 1: []}
            for k in range(KC):
                t = wk_pool.tile([128, N // 2], BF16, tag="es", name="xqt", bufs=ES_BUFS)
                nc.sync.dma_start(t[:], qT[128 * k:128 * (k + 1), 0:1024])
                nc.gpsimd.dma_start(wq_sb[k][:], wq[128 * k:128 * (k + 1), :])
                q_half[0].append(t)
            for k in range(KC):
                nc.sync.dma_start(
                    k_tiles[k][:, 1024:2048], kT[128 * k:128 * (k + 1), 1024:2048]
                )
            for k in range(KC):
                t = wk_pool.tile([128, N // 2], BF16, tag="es", name="xqt", bufs=ES_BUFS)
                nc.sync.dma_start(t[:], qT[128 * k:128 * (k + 1), 1024:2048])
                q_half[1].append(t)

            # ---- projection emitters --------------------------------------
            qp_sb = [pp.tile([128, N], BF16, tag=f"qp{m}", name=f"qp{m}") for m in range(2)]
            kp_sb = [pp.tile([128, N], BF16, tag=f"kp{m}", name=f"kp{m}") for m in range(2)]
            vpa = [pp.tile([128, H_LOC * 65], BF16, tag=f"vpa{j}", name=f"vpa{j}") for j in range(KT)]

            def gen_xproj(w_sb, rhs_of, p_sb, m):
                """Generator: one projection (16 mms + copies per cc-group),
                yielding after each matmul so it can interleave with the
                softmax stream. PSUM on the 1-bank "epi" tag."""
                for cc in (0, 2):
                    ps2 = [
                        psum.tile([128, 512], F32, tag="epi", name="pps", bufs=2)
                        for _ in range(2)
                    ]
                    for k in range(KC):
                        for ci in range(2):
                            nc.tensor.matmul(
                                ps2[ci][:],
                                lhsT=w_sb[k][:, 128 * m:128 * (m + 1)],
                                rhs=rhs_of(k, cc + ci),
                                start=(k == 0),
                                stop=(k == KC - 1),
                            )
                            yield
                    for ci in range(2):
                        c = cc + ci
                        nc.vector.tensor_copy(
                            p_sb[m][:, 512 * c:512 * (c + 1)], ps2[ci][:]
                        )

            def gen_kproj(m):
                return gen_xproj(
                    wk_sb, lambda k, c: k_tiles[k][:, 512 * c:512 * (c + 1)],
                    kp_sb, m,
                )

            def gen_qproj(m):
                return gen_xproj(
                    wq_sb,
                    lambda k, c: q_half[c // 2][k][:, 512 * (c % 2):512 * (c % 2 + 1)],
                    qp_sb, m,
                )

            def gen_vproj(m):
                return gen_xproj(
                    wv_sb, lambda k, c: v_tiles[k][:, 512 * c:512 * (c + 1)],
                    vpt_sb, m,
                )

            slices = [(j, h) for j in range(KT) for h in range(2)]

            def emit_s_exp(m, c, b0):
                """One S+exp batch (2 ktile-slices, heads interleaved so the
                K=64 S-matmuls pack pairwise in the PE array)."""
                batch = slices[b0:b0 + 2]
                w = 512 * len(batch)
                sp = psum.tile([128, 1024], F32, tag="sp", name="sp", bufs=2)
                es = wk_pool.tile([128, 1024], BF16, tag="es", name="es", bufs=ES_BUFS)
                for s, (j, h) in enumerate(batch):
                    p0 = 64 * h
                    nc.tensor.matmul(
                        sp[:, 512 * s:512 * (s + 1)],
                        lhsT=kp_sb[m][p0:p0 + 64, 128 * j:128 * (j + 1)],
                        rhs=qp_sb[m][p0:p0 + 64, 512 * c:512 * (c + 1)],
                        start=True,
                        stop=True,
                    )
                nc.scalar.activation(
                    es[:, 0:w], sp[:, 0:w],
                    mybir.ActivationFunctionType.Exp, scale=SCALE,
                )
                return es

            def emit_o(m, b0, es, ot_ps):
                for s, (j, h) in enumerate(slices[b0:b0 + 2]):
                    hl = 2 * m + h
                    nc.tensor.matmul(
                        ot_ps[h][:],
                        lhsT=vpa[j][:, 65 * hl:65 * hl + 65],
                        rhs=es[:, 512 * s:512 * (s + 1)],
                        start=(j == 0),
                        stop=(j == KT - 1),
                    )

            def emit_vpa(j, m):
                tp = psum.tile([128, 128], BF16, tag="epi", name="tp", bufs=2)
                nc.tensor.transpose(
                    tp[:], vpt_sb[m][:, 128 * j:128 * (j + 1)], ident[:]
                )
                dst = vpa[j][:, 130 * m:130 * (m + 1)].rearrange(
                    "p (h e) -> p h e", e=65
                )[:, :, 0:64]
                nc.vector.tensor_copy(dst, tp[:].rearrange("p (h e) -> p h e", e=64))

            def new_ot_ps():
                return [
                    psum.tile([65, 512], F32, tag="otps", name=f"otps{h}", bufs=2)
                    for h in range(2)
                ]

            def unload_pair(m, ot_ps, pair_tile, den_c, tail=False):
                # one 65-row copy per head (O rows + bf16 denominator row)
                # releases the PSUM accumulators after just two ops; in the
                # tail these ride the otherwise-idle ScalarE
                stage_e = wk_pool.tile([65, 512], BF16, tag="stge", name="stge", bufs=2)
                stage_o = wk_pool.tile([65, 512], BF16, tag="stgo", name="stgo", bufs=2)
                eng = nc.scalar if tail else nc.vector
                if tail:
                    eng.copy(stage_e[:], ot_ps[0][:])
                    eng.copy(stage_o[:], ot_ps[1][:])
                else:
                    nc.vector.tensor_copy(stage_e[:], ot_ps[0][:])
                    nc.vector.tensor_copy(stage_o[:], ot_ps[1][:])
                nc.vector.tensor_copy(pair_tile[0:64, :], stage_e[0:64, :])
                nc.sync.dma_start(pair_tile[64:128, :], stage_o[0:64, :])
                nc.sync.dma_start(den_c[2 * m:2 * m + 1, :], stage_e[64:65, :])
                nc.sync.dma_start(den_c[2 * m + 1:2 * m + 2, :], stage_o[64:65, :])

            def emit_epilogue(c, ot_sb, den_c, tail=False):
                """normalize (recip -> mask-matmul broadcast -> multiply) and
                the Wo projection for one n_q chunk."""
                den_f = wk_pool.tile([4, 512], F32, tag="denf", name="denf", bufs=2)
                recip_f = wk_pool.tile([4, 512], F32, tag="recf", name="recf", bufs=2)
                recip_b = wk_pool.tile([4, 512], BF16, tag="recb", name="recb", bufs=2)
                nc.vector.tensor_copy(den_f[:], den_c[:])
                nc.vector.reciprocal_approx_fast(recip_f[:], den_f[:])
                nc.vector.tensor_copy(recip_b[:], recip_f[:])
                for m in range(2):
                    bc = psum.tile([128, 512], F32, tag="epi", name="bc", bufs=2)
                    nc.tensor.matmul(
                        bc[:], lhsT=e_mask[m], rhs=recip_b[:], start=True, stop=True,
                    )
                    nc.vector.tensor_mul(ot_sb[m][:], ot_sb[m][:], bc[:])
                for s in range(4):
                    for dch in range(2):
                        ops = psum.tile([128, 512], F32, tag="epi", name="op", bufs=2)
                        for m in range(2):
                            nc.tensor.matmul(
                                ops[:],
                                lhsT=ot_sb[m][:, 128 * s:128 * (s + 1)],
                                rhs=wo_sb[m][:, 512 * dch:512 * (dch + 1)],
                                start=(m == 0),
                                stop=(m == 1),
                            )
                        o_sb = wk_pool.tile([128, 512], F32, tag="osb", name="osb", bufs=2)
                        if tail:
                            nc.scalar.copy(o_sb[:], ops[:])
                        else:
                            nc.vector.tensor_copy(o_sb[:], ops[:])
                        r0 = 512 * c + 128 * s
                        nc.sync.dma_start(
                            out[r0:r0 + 128, 512 * dch:512 * (dch + 1)], o_sb[:]
                        )

            # ---- emission schedule: 2-deep software pipeline ---------------
            # unit u = (pair m=u%2, chunk c=u//2). Unit u's S+exp batches
            # interleave with unit (u-2)'s O-pass; the prologue units carry
            # the pair-1 projections / V projection / PE transposes instead.
            # v inputs + remaining weights
            v_tiles = []
            for k in range(KC):
                t = xkv.tile([128, N], BF16, tag="xt", name="xvt")
                nc.sync.dma_start(t[:], vT[128 * k:128 * (k + 1), :])
                nc.gpsimd.dma_start(wv_sb[k][:], wv[128 * k:128 * (k + 1), :])
                v_tiles.append(t)
            for m in range(2):
                nc.gpsimd.dma_start(wo_sb[m][:], wo[128 * m:128 * (m + 1), :])

            for _ in gen_kproj(0):
                pass
            for _ in gen_qproj(0):
                pass

            units = [(u % 2, u // 2) for u in range(2 * NQC)]
            es_held = {}     # u -> list of es tiles awaiting the O pass
            ot_ps_of = {}
            ot_sb_of = {}
            den_of = {}

            def feeder_prologue0():
                # pair-1 K/Q projections: 64 matmuls
                for g in (gen_kproj(1), gen_qproj(1)):
                    for _ in g:
                        yield

            def feeder_prologue1():
                # transposed V projection: 64 matmuls
                for g in (gen_vproj(0), gen_vproj(1)):
                    for _ in g:
                        yield
                for j in range(KT):
                    nc.vector.memset(vpa[j][:], 1.0)

            for u, (m, c) in enumerate(units):
                # start-of-unit bookkeeping
                if u % 2 == 0:
                    den_of[c] = wk_pool.tile([4, 512], BF16, tag="den", name="den", bufs=2)
                ot_sb_of[u] = wk_pool.tile(
                    [128, 512], BF16, tag=f"ot{u}", name=f"ot{u}", bufs=1
                )
                if u == 0:
                    feeder = feeder_prologue0()
                elif u == 1:
                    feeder = feeder_prologue1()
                else:
                    feeder = None
                if u >= 2:
                    ot_ps_of[u - 2] = new_ot_ps()
                es_held[u] = []
                for b in range(KT):
                    if feeder is not None:
                        for _ in range(4):
                            next(feeder, None)
                    elif u >= 2:
                        up = u - 2
                        if up == 0:
                            emit_vpa(b, 0)
                            emit_vpa(b, 1)
                        emit_o(units[up][0], 2 * b, es_held[up][b], ot_ps_of[up])
                    es_held[u].append(emit_s_exp(m, c, 2 * b))
                if feeder is not None:
                    for _ in feeder:
                        pass
                if u >= 2:
                    up = u - 2
                    mp, cp = units[up]
                    unload_pair(mp, ot_ps_of[up], ot_sb_of[up], den_of[cp])
                    del es_held[up]
                    if mp == 1:
                        emit_epilogue(
                            cp, [ot_sb_of[2 * cp], ot_sb_of[2 * cp + 1]],
                            den_of[cp], tail=(cp >= 2),
                        )

            # drain: O-passes for the last two units
            for up in (2 * NQC - 2, 2 * NQC - 1):
                mp, cp = units[up]
                ot_ps_of[up] = new_ot_ps()
                for b in range(KT):
                    emit_o(mp, 2 * b, es_held[up][b], ot_ps_of[up])
                unload_pair(mp, ot_ps_of[up], ot_sb_of[up], den_of[cp], tail=True)
            emit_epilogue(
                NQC - 1, [ot_sb_of[2 * NQC - 2], ot_sb_of[2 * NQC - 1]],
                den_of[NQC - 1], tail=True,
            )

    nc.compile()
    return nc


_NC_CACHE = None


def _get_nc():
    global _NC_CACHE
    if _NC_CACHE is None:
        _NC_CACHE = _build_nc()
    return _NC_CACHE


def kernel(q, k, v, Wq, Wk, Wv, Wo):
    q = np.asarray(q, dtype=np.float32)
    k = np.asarray(k, dtype=np.float32)
    v = np.asarray(v, dtype=np.float32)
    Wq = np.asarray(Wq, dtype=np.float32)
    Wk = np.asarray(Wk, dtype=np.float32)
    Wv = np.asarray(Wv, dtype=np.float32)
    Wo = np.asarray(Wo, dtype=np.float32)

    qT = [np.ascontiguousarray(q[g].T).astype(NPBF16) for g in range(B)]
    kT = [np.ascontiguousarray(k[g].T).astype(NPBF16) for g in range(B)]
    vT = [np.ascontiguousarray(v[g].T).astype(NPBF16) for g in range(B)]
    wq_b = Wq.astype(NPBF16)
    wk_b = Wk.astype(NPBF16)
    wv_b = Wv.astype(NPBF16)
    wo_b = Wo.astype(NPBF16)
    emask = np.zeros((4, 256), NPBF16)
    for m in range(2):
        emask[2 * m, 128 * m:128 * m + 64] = 1
        emask[2 * m + 1, 128 * m + 64:128 * m + 128] = 1

    in_maps = []
    for c in range(8):
        g, t = c // 4, c % 4
        sl = slice(INNER * t, INNER * (t + 1))
        in_maps.append({
            "qT": qT[g],
            "kT": kT[g],
            "vT": vT[g],
            "wq": np.ascontiguousarray(wq_b[:, sl]),
            "wk": np.ascontiguousarray(wk_b[:, sl]),
            "wv": np.ascontiguousarray(wv_b[:, sl]),
            "wo": np.ascontiguousarray(wo_b[sl, :]),
            "emask": emask,
        })

    nc = _get_nc()
    res = run_bass_kernel_spmd(nc, in_maps, core_ids=list(range(8)))

    out = np.empty((B, N, D), np.float32)
    for g in range(B):
        acc = res.results[4 * g]["out"].astype(np.float32)
        for t in range(1, 4):
            acc = acc + res.results[4 * g + t]["out"]
        out[g] = acc
    return out
